# revision 1
# baseline (speedup 1.0000x reference)
import sys

sys.path.insert(0, "/opt/trn_rl_repo")

import numpy as np
import ml_dtypes

import concourse.bass as bass
import concourse.bacc as bacc
import concourse.tile as tile
from concourse.bass_utils import run_bass_kernel_spmd
from concourse import mybir

B, L, D, H = 2, 2048, 1024, 16
DH = 64          # dim per head
HPC = 4          # heads per core
CPC = HPC * DH   # feature cols per core = 256
NCORES = 8

MM_DT = "bfloat16"
NP_MM = ml_dtypes.bfloat16 if MM_DT == "bfloat16" else np.float32

_CACHE = {}


def build_nc(mm_dt: str):
    nc = bacc.Bacc()
    mm_dt = mybir.dt(mm_dt)
    fp32 = mybir.dt.float32

    xq = nc.declare_dram_parameter("xq", (D, L), mm_dt, isOutput=False)   # q[b].T
    xk = nc.declare_dram_parameter("xk", (D, L), mm_dt, isOutput=False)   # k[b].T
    xv = nc.declare_dram_parameter("xv", (D, L), mm_dt, isOutput=False)   # v[b].T
    wq = nc.declare_dram_parameter("wq", (D, CPC), mm_dt, isOutput=False)
    wk = nc.declare_dram_parameter("wk", (D, CPC), mm_dt, isOutput=False)
    wv = nc.declare_dram_parameter("wv", (D, CPC), mm_dt, isOutput=False)
    wo = nc.declare_dram_parameter("wo", (CPC, D), mm_dt, isOutput=False)
    # packed biases: cols 0:2 = bq (cc0,cc1), cols 2:4 = bk -- single fat
    # descriptor instead of 256 4-byte ones
    bqk = nc.declare_dram_parameter("bqk", (128, 4), fp32, isOutput=False)
    y = nc.declare_dram_parameter("y", (L, D), mm_dt, isOutput=True)      # partial out (bf16)

    from contextlib import ExitStack

    with ExitStack() as es:
        tc = es.enter_context(tile.TileContext(nc))
        # NOTE: bufs are per named tag
        xt_pool = es.enter_context(tc.tile_pool(name="xt", bufs=1))     # 3 tags [128,8,2048]
        w_pool = es.enter_context(tc.tile_pool(name="w", bufs=1))       # 3 tags [128,8,256]
        wo_pool = es.enter_context(tc.tile_pool(name="wo", bufs=1))     # 2 tags [128,1024]
        bias_pool = es.enter_context(tc.tile_pool(name="bias", bufs=1))
        qt_pool = es.enter_context(tc.tile_pool(name="qt", bufs=1))     # 2 tags [128,2048]
        kt_pool = es.enter_context(tc.tile_pool(name="kt", bufs=1))
        vn_pool = es.enter_context(tc.tile_pool(name="vn", bufs=1))     # [128,16,4,65]
        pt_pool = es.enter_context(tc.tile_pool(name="pt", bufs=3))     # [128,2048] bf16
        zr_pool = es.enter_context(tc.tile_pool(name="zr", bufs=2))
        zbs_pool = es.enter_context(tc.tile_pool(name="zbs", bufs=2))
        ot_pool = es.enter_context(tc.tile_pool(name="ot", bufs=1))     # 2 tags [128,2048]
        y_pool = es.enter_context(tc.tile_pool(name="ysb", bufs=4))     # [128,1024] bf16
        psA = es.enter_context(tc.tile_pool(name="psA", bufs=2, space="PSUM"))   # 2 banks
        psS = es.enter_context(tc.tile_pool(name="psS", bufs=1, space="PSUM"))   # [128,2048] = 4 banks
        psOT = es.enter_context(tc.tile_pool(name="psOT", bufs=1, space="PSUM"))  # [65,1024] = 2 banks

        # ---- input DMAs (queue order == consumption order) --------------
        wk_sb = w_pool.tile([128, 8, CPC], mm_dt, name="wk")
        wq_sb = w_pool.tile([128, 8, CPC], mm_dt, name="wq")
        wv_sb = w_pool.tile([128, 8, CPC], mm_dt, name="wv")
        xk_sb = xt_pool.tile([128, 8, L], mm_dt, name="xk")
        xq_sb = xt_pool.tile([128, 8, L], mm_dt, name="xq")
        xv_sb = xt_pool.tile([128, 8, L], mm_dt, name="xv")
        wk_r = wk.rearrange("(dc p) c -> p dc c", p=128)
        wq_r = wq.rearrange("(dc p) c -> p dc c", p=128)
        wv_r = wv.rearrange("(dc p) c -> p dc c", p=128)
        xk_r = xk.rearrange("(dc p) c -> p dc c", p=128)
        xq_r = xq.rearrange("(dc p) c -> p dc c", p=128)
        xv_r = xv.rearrange("(dc p) c -> p dc c", p=128)

        # single sync DMA ring (multiple rings share the same ~350 GB/s and
        # only starve the critical path); strict first-needed-first order
        nc.sync.dma_start(out=wk_sb[:, 0:4, :], in_=wk_r[:, 0:4, :])
        nc.sync.dma_start(out=xk_sb[:, 0:2, 0:512], in_=xk_r[:, 0:2, 0:512])
        nc.sync.dma_start(out=xk_sb[:, 2:4, 0:512], in_=xk_r[:, 2:4, 0:512])
        nc.sync.dma_start(out=wk_sb[:, 4:8, :], in_=wk_r[:, 4:8, :])
        nc.sync.dma_start(out=xk_sb[:, 4:8, 0:512], in_=xk_r[:, 4:8, 0:512])
        bias_sb = bias_pool.tile([128, 4], fp32, name="bqk")
        nc.sync.dma_start(out=bias_sb, in_=bqk[:, :])
        nc.sync.dma_start(out=wq_sb[:, 0:4, :], in_=wq_r[:, 0:4, :])
        nc.sync.dma_start(out=xq_sb[:, 0:4, 0:512], in_=xq_r[:, 0:4, 0:512])
        nc.sync.dma_start(out=wq_sb[:, 4:8, :], in_=wq_r[:, 4:8, :])
        nc.sync.dma_start(out=xq_sb[:, 4:8, 0:512], in_=xq_r[:, 4:8, 0:512])
        nc.sync.dma_start(out=wv_sb, in_=wv_r)
        nc.sync.dma_start(out=xv_sb[:, :, 0:512], in_=xv_r[:, :, 0:512])
        for ch in range(1, 4):
            sl = slice(512 * ch, 512 * ch + 512)
            nc.sync.dma_start(out=xk_sb[:, :, sl], in_=xk_r[:, :, sl])
            nc.sync.dma_start(out=xq_sb[:, :, sl], in_=xq_r[:, :, sl])
            nc.sync.dma_start(out=xv_sb[:, :, sl], in_=xv_r[:, :, sl])

        wo_sb = []
        for cc in range(2):
            t = wo_pool.tile([128, D], mm_dt, name=f"wo{cc}")
            nc.sync.dma_start(out=t, in_=wo[cc * 128:(cc + 1) * 128, :])
            wo_sb.append(t)

        # ---- persistent SBUF staging ------------------------------------
        qt_sb = [qt_pool.tile([128, L], mm_dt, name=f"qt{i}") for i in range(2)]
        kt_sb = [kt_pool.tile([128, L], mm_dt, name=f"kt{i}") for i in range(2)]
        # V natural layout: [128(lt-part), 16 lt, 4 head, 65] (col 64 = ones)
        v_sb = vn_pool.tile([128, 16, 4, 65], mm_dt)
        nc.vector.memset(v_sb[:, :, :, 64:65], 1.0)
        # one-time [128,128] causal triangle (tri[p,f] = f >= p); later applied
        # on DVE via tensor_mul so gpsimd only ever runs partition_broadcast
        tri_sb = bias_pool.tile([128, 128], mm_dt, name="tri")
        nc.vector.memset(tri_sb, 1.0)
        nc.gpsimd.affine_select(
            out=tri_sb,
            in_=tri_sb,
            compare_op=mybir.AluOpType.is_ge,
            fill=0.0,
            base=0,
            channel_multiplier=-1,
            pattern=[[1, 128]],
        )
        # per-(cc, g4) O tiles: a single [128, L] tile per cc makes every C
        # matmul's transposed weight-read conservatively depend on ALL later
        # divisions (whole-tile tracking), serializing the tail
        ot_sb = [[ot_pool.tile([128, 512], mm_dt, name=f"ot{i}g{g}")
                  for g in range(4)] for i in range(2)]
        y_view = y.rearrange("(lt p) c -> p lt c", p=128)

        # ---- filler units (one unit ~= 2 matmuls, popped into exp gaps) -
        def qk_units(dst, x_sb, w_sb, bidx, lg, cc):
            state = {}

            def mk(i):
                def f():
                    if i == 0:
                        state["ps"] = psA.tile([128, 512], fp32, name="ps")
                    ps = state["ps"]
                    for dc in (2 * i, 2 * i + 1):
                        nc.tensor.matmul(
                            ps,
                            w_sb[:, dc, cc * 128:(cc + 1) * 128],
                            x_sb[:, dc, lg * 512:(lg + 1) * 512],
                            start=(dc == 0),
                            stop=(dc == 7),
                        )
                    if i == 3:
                        nc.vector.tensor_scalar_add(
                            out=dst[cc][:, lg * 512:(lg + 1) * 512],
                            in0=ps,
                            scalar1=bias_sb[:, bidx:bidx + 1],
                        )
                return f

            return [(("qk", lg, cc), mk(i)) for i in range(4)]

        def v_units(lt):
            state = {}

            def mk(i):
                def f():
                    if i == 0:
                        state["ps"] = psA.tile([128, CPC], fp32, name="ps")
                    ps = state["ps"]
                    for dc in (2 * i, 2 * i + 1):
                        nc.tensor.matmul(
                            ps,
                            xv_sb[:, dc, lt * 128:(lt + 1) * 128],
                            wv_sb[:, dc, :],
                            start=(dc == 0),
                            stop=(dc == 7),
                        )
                    if i == 3:
                        nc.vector.tensor_copy(
                            out=v_sb[:, lt, :, 0:64],
                            in_=ps.rearrange("p (h d) -> p h d", d=64),
                        )
                return f

            return [(("v", lt), mk(i)) for i in range(4)]

        def c_units(g4, act_copy=False):
            units = []
            for li in range(4):
                lt = g4 * 4 + li
                for dg in range(2):
                    def f(lt=lt, dg=dg):
                        ps = psA.tile([128, 512], fp32, name="ps")
                        for cc2 in range(2):
                            nc.tensor.matmul(
                                ps,
                                ot_sb[cc2][lt // 4][:, (lt % 4) * 128:
                                                    (lt % 4 + 1) * 128],
                                wo_sb[cc2][:, dg * 512:(dg + 1) * 512],
                                start=(cc2 == 0),
                                stop=(cc2 == 1),
                            )
                        yt = y_pool.tile([128, 512], mm_dt, name="yh")
                        if act_copy:
                            # tail units alternate ACT/DVE copies: ACT is idle
                            # once the exps are done and the DVE FIFO must not
                            # back up behind the final division chain
                            nc.scalar.activation(
                                out=yt, in_=ps,
                                func=mybir.ActivationFunctionType.Copy,
                                bias=0.0,
                            )
                        else:
                            nc.vector.tensor_copy(out=yt, in_=ps)
                        nc.sync.dma_start(
                            out=y_view[:, lt, dg * 512:(dg + 1) * 512],
                            in_=yt,
                        )
                    units.append((None, f))
            return units

        queue = []
        reserve = []

        def pump(n):
            k = 0
            while k < n and queue:
                queue.pop(0)[1]()
                k += 1

        def need(tag):
            # emit from the front until no unit with this tag remains
            while any(t == tag for t, _ in queue):
                queue.pop(0)[1]()

        # ---- prologue: Q/K projections for lg=0, cc=0 only (serial);
        # cc=1 units go to the queue so S(g4=0, hp=0) starts ~5us earlier
        for _, f in qk_units(kt_sb, xk_sb, wk_sb, 2, 0, 0):
            f()
        for _, f in qk_units(qt_sb, xq_sb, wq_sb, 0, 0, 0):
            f()
        queue += v_units(0) + v_units(1)
        queue += qk_units(kt_sb, xk_sb, wk_sb, 3, 0, 1)
        queue += qk_units(qt_sb, xq_sb, wq_sb, 1, 0, 1)
        queue += v_units(2) + v_units(3)

        EXP = mybir.ActivationFunctionType.Exp

        # ---- main attention loop: head pairs, row-tiled S ---------------
        def emit_S(g4, cc, k0):
            # S staging for the head pair lives in ONE [128,2048] PSUM tile
            # (4 banks): cols idx*1024 + j*512 hold (head idx, key tile k0+j).
            # One wide ACTIVATE then exps both heads at once, halving the
            # per-instruction ACT overhead.
            diag = (k0 // 4 == g4)
            st = psS.tile([128, 2048], fp32, name="st")
            pt = pt_pool.tile([128, 2048], mm_dt, name="pt")
            # S: row-tiled head pair (h0 rows 0:64, h1 rows 64:128)
            for j in range(2):
                kt = k0 + j
                off = 128 * (kt % 4) if diag else 0
                base = j * 512
                for idx in range(2):
                    r = idx * 64
                    nc.tensor.matmul(
                        st[:, idx * 1024 + base + off:idx * 1024 + base + 512],
                        kt_sb[cc][r:r + 64, kt * 128:(kt + 1) * 128],
                        qt_sb[cc][r:r + 64, g4 * 512 + off:(g4 + 1) * 512],
                        start=True,
                        stop=True,
                    )
            # exp (+ causal mask on diag tiles)
            if not diag:
                nc.scalar.activation(out=pt, in_=st, func=EXP, scale=0.125)
            else:
                st3 = st.rearrange("p (i c) -> p i c", i=2)
                pt3 = pt.rearrange("p (i c) -> p i c", i=2)
                for j in range(2):
                    kt = k0 + j
                    off = 128 * (kt % 4)
                    base = j * 512
                    nc.scalar.activation(
                        out=pt3[:, :, base + off:base + 512],
                        in_=st3[:, :, base + off:base + 512],
                        func=EXP,
                        scale=0.125,
                    )
                    # mask only the 128-wide staircase strip (cols below
                    # base+off are never read by the trimmed PV); tensor_mul
                    # on DVE keeps gpsimd free for the division broadcasts
                    strip = pt3[:, :, base + off:base + off + 128]
                    nc.vector.tensor_mul(
                        out=strip,
                        in0=strip,
                        in1=tri_sb[:, None, :].broadcast_to([128, 2, 128]),
                    )
            return pt

        def emit_PV(g4, hp, k0, pt, ot_ps, nkt):
            diag = (k0 // 4 == g4)
            for idx in range(2):
                h = 2 * hp + idx
                for j in range(2):
                    kt = k0 + j
                    if g4 == kt // 4:
                        need(("v", kt))
                    off = 128 * (kt % 4) if diag else 0
                    base = j * 512
                    nc.tensor.matmul(
                        ot_ps[idx][:, off:512],
                        v_sb[:, kt, h, :],
                        pt[:, idx * 1024 + base + off:idx * 1024 + base + 512],
                        start=(kt == 0),
                        stop=(kt == nkt - 1),
                    )

        for g4 in range(4):
            # refill the filler queue (ordered by first-need time);
            # all C groups are held back to g4=3 where filler demand peaks
            if g4 == 0:
                queue += qk_units(kt_sb, xk_sb, wk_sb, 2, 1, 0)
                queue += qk_units(kt_sb, xk_sb, wk_sb, 3, 1, 1)
                queue += qk_units(qt_sb, xq_sb, wq_sb, 0, 1, 0)
                queue += qk_units(qt_sb, xq_sb, wq_sb, 1, 1, 1)
            elif g4 == 1:
                queue += v_units(4) + v_units(5) + v_units(6) + v_units(7)
                queue += qk_units(kt_sb, xk_sb, wk_sb, 2, 2, 0)
                queue += qk_units(kt_sb, xk_sb, wk_sb, 3, 2, 1)
                queue += qk_units(qt_sb, xq_sb, wq_sb, 0, 2, 0)
                queue += qk_units(qt_sb, xq_sb, wq_sb, 1, 2, 1)
            elif g4 == 2:
                queue += v_units(8) + v_units(9) + v_units(10) + v_units(11)
                queue += qk_units(kt_sb, xk_sb, wk_sb, 2, 3, 0)
                queue += qk_units(kt_sb, xk_sb, wk_sb, 3, 3, 1)
                queue += qk_units(qt_sb, xq_sb, wq_sb, 0, 3, 0)
                queue += qk_units(qt_sb, xq_sb, wq_sb, 1, 3, 1)
            elif g4 == 3:
                queue += v_units(12) + v_units(13) + v_units(14) + v_units(15)
                queue += c_units(0) + c_units(1)
                queue += c_units(2)[:4]
                reserve.extend(c_units(2, act_copy=True)[4:])

            nkt = 4 * g4 + 4
            for hp in range(2):
                cc = hp
                # this head-pair's Q/K projection columns must be emitted
                # before any S matmul reads them
                need(("qk", g4, cc))
                ot_ps = [
                    psOT.tile([65, 512], fp32, name="ot0"),
                    psOT.tile([65, 512], fp32, name="ot1"),
                ]
                # software pipeline: S/exp run one key-pair ahead of PV so
                # the ACT exp stream never waits on the PV chain
                pend = None
                pts = emit_S(g4, cc, 0)
                pump(2)
                for k0 in range(2, nkt, 2):
                    pend = pts
                    pts = emit_S(g4, cc, k0)
                    emit_PV(g4, hp, k0 - 2, pend, ot_ps, nkt)
                    pump(3)
                # last key-pair: emit each head's PV then immediately its
                # division so the DVE chain starts ~1us earlier. Divisions are
                # emitted WITHOUT pumping in between: a pumped PE-dependent
                # v-copy in the DVE FIFO ahead of the recip chain transitively
                # stalls the next head-pair (near-deadlock, ~9us each).
                # NB: reciprocal_approx_fast reading PSUM directly is
                # silently wrong; bounce the row through SBUF first.
                diag_l = ((nkt - 2) // 4 == g4)
                for idx in range(2):
                    h = 2 * hp + idx
                    for j in range(2):
                        kt = nkt - 2 + j
                        if g4 == kt // 4:
                            need(("v", kt))
                        off = 128 * (kt % 4) if diag_l else 0
                        base = j * 512
                        nc.tensor.matmul(
                            ot_ps[idx][:, off:512],
                            v_sb[:, kt, h, :],
                            pts[:, idx * 1024 + base + off:idx * 1024 + base + 512],
                            start=(kt == 0),
                            stop=(kt == nkt - 1),
                        )
                for idx in range(2):
                    zrow = zr_pool.tile([1, 512], fp32, name=f"zrow{idx}")
                    nc.vector.tensor_copy(out=zrow, in_=ot_ps[idx][64:65, :])
                    zrc = zr_pool.tile([1, 512], fp32, name=f"zr{idx}")
                    nc.vector.reciprocal_approx_fast(out=zrc, in_=zrow)
                    zb = zbs_pool.tile([64, 512], fp32, name=f"zb{idx}")
                    nc.gpsimd.partition_broadcast(out_ap=zb, in_ap=zrc)
                    nc.vector.tensor_mul(
                        out=ot_sb[cc][g4][idx * 64:idx * 64 + 64, :],
                        in0=ot_ps[idx][0:64, :],
                        in1=zb,
                    )
                pump(4)

        # ---- tail: reserved C units bridge the last division window so
        # the PE never idles >3.4us (HAM would re-throttle the whole tail);
        # then C(3) in two wide 4-bank groups
        for _, f in reserve:
            f()
        while queue:
            queue.pop(0)[1]()
        for half in range(2):
            ps = psS.tile([128, 2048], fp32, name="st")
            for li in range(2):
                lt = 12 + half * 2 + li
                for dg in range(2):
                    for cc2 in range(2):
                        nc.tensor.matmul(
                            ps[:, li * 1024 + dg * 512:li * 1024 + (dg + 1) * 512],
                            ot_sb[cc2][3][:, (lt % 4) * 128:(lt % 4 + 1) * 128],
                            wo_sb[cc2][:, dg * 512:(dg + 1) * 512],
                            start=(cc2 == 0),
                            stop=(cc2 == 1),
                        )
            yt = y_pool.tile([128, 2048], mm_dt, name="yt")
            # split the PSUM->SBUF cast across DVE and ACT so the two wide
            # C groups don't serialize on one 2.3us fp32 copy
            nc.vector.tensor_copy(out=yt[:, 0:1024], in_=ps[:, 0:1024])
            nc.scalar.activation(
                out=yt[:, 1024:2048], in_=ps[:, 1024:2048],
                func=mybir.ActivationFunctionType.Copy,
                bias=0.0,
            )
            for li in range(2):
                lt = 12 + half * 2 + li
                nc.sync.dma_start(
                    out=y_view[:, lt, :],
                    in_=yt[:, li * 1024:(li + 1) * 1024],
                )

    nc.compile()
    return nc


def _get_nc(mm_dt: str):
    if mm_dt not in _CACHE:
        _CACHE[mm_dt] = build_nc(mm_dt)
    return _CACHE[mm_dt]


def kernel(q, k, v, mask, Wq, bq, Wk, bk, Wv, bv, Wo, bo, _trace=False):
    nc = _get_nc(MM_DT)

    in_maps = []
    for c in range(NCORES):
        b = c // 4
        g = c % 4
        s = slice(g * CPC, (g + 1) * CPC)
        bq_s = np.ascontiguousarray(bq[s]).reshape(2, 128).T
        bk_s = np.ascontiguousarray(bk[s]).reshape(2, 128).T
        in_maps.append({
            "xq": np.ascontiguousarray(q[b].T).astype(NP_MM),
            "xk": np.ascontiguousarray(k[b].T).astype(NP_MM),
            "xv": np.ascontiguousarray(v[b].T).astype(NP_MM),
            "wq": np.ascontiguousarray(Wq[:, s]).astype(NP_MM),
            "wk": np.ascontiguousarray(Wk[:, s]).astype(NP_MM),
            "wv": np.ascontiguousarray(Wv[:, s]).astype(NP_MM),
            "wo": np.ascontiguousarray(Wo[s, :]).astype(NP_MM),
            "bqk": np.ascontiguousarray(
                np.concatenate([bq_s, bk_s], axis=1)).astype(np.float32),
        })

    res = run_bass_kernel_spmd(nc, in_maps, list(range(NCORES)), trace=_trace)

    # host gather: out[b] = sum_g y_core(b,g) + (bo + bv @ Wo)
    const = (bo + bv.astype(np.float64) @ Wo.astype(np.float64)).astype(np.float64)
    out = np.zeros((B, L, D), np.float64)
    for c in range(NCORES):
        out[c // 4] += res.results[c]["y"].astype(np.float64)
    out += const[None, None, :]
    kernel.last_exec_time_ns = res.exec_time_ns
    return out.astype(np.float32)

